# revision 2
# baseline (speedup 1.0000x reference)
import sys

sys.path.insert(0, "/opt/trn_rl_repo")

import numpy as np

import concourse.bacc as bacc
import concourse.bass as bass
import concourse.mybir as mybir
import concourse.tile as tile
from concourse.bass_utils import run_bass_kernel_spmd

F32 = mybir.dt.float32
F16 = mybir.dt.float16

N, M, G, A, H = 20000, 48, 16, 64, 16
NCORES = 8
NL = N // NCORES  # 2500 atoms per core
NLP = 2560  # padded per-core atoms (5 groups of 512)
NPAIR = NLP // 2  # 1280 atom pairs (even slot j, odd slot NLP/2+j)
NCHK = 32  # pairs per stage-1 chunk
CCW = NCHK * 64 * 3  # staging cols per chunk: G-region 4096 + A-region 2048
NB2 = 512  # group size (atoms) = 256 pairs
NFL = 8  # pairs per psum flush
NST = 4  # staging buffers (memset once; DMA rewrites data rows only)

_nc_cache = {}


def _build(nlp=NLP, sim=False, reps=1):
    """Per-core Bass program, software-pipelined, all matmuls fp16.

    Stage 1 (one MM per atom PAIR): staging tile [128, 6144] per 32-pair
    chunk: G-region 0:4096 (pair j at cols j*128, block-diagonal
    [gblk_e | gblk_o] over partition halves 0-47 / 64-111, zeros shipped
    from host; col order within a pair = (sec4, eo2, g16)); A-region
    4096:6144 (pair j at 4096+j*64, rows 0-47 = a_e, 64-111 = a_o).
    MM: lhsT = 128-col G-pair block (FWL), rhs = 64-col a block ->
    psum1 partitions = (sec, eo, g), cols = (pair, ch). Evac transposes
    cols to ch-major vbig [128 = (sec,eo,g), 64 ch x 256 pairs].

    Stage 2 (t stationary, agh moving): per (B: 128-pair block, C: 8-ch
    group): 32 MMs: lhsT = vbig[32s:32s+32, ch*256+B*128 :+128] (contig,
    FWL), rhs = aghw[32s:32s+32, ch*32:+32] = blockdiag[[agh,0],[0,agh]]
    (band s=0 holds the identity for the S passthrough) -> psum2
    [128 = pairs, (c8, s4, eo2, 16)]. ACT squares the d bands (s=1..3)
    into f32 sq (c, eo, h, s); DVE reduce over s -> vout V region; ACT
    copies the S band -> vout S region. One output DMA per B:
    [128 pairs, (eo, 2048)] f32 -> out rows eo*1280 + pair.
    """
    assert nlp % NB2 == 0
    nc = bacc.Bacc("TRN2", target_bir_lowering=False)
    nchunk = nlp // 2 // NCHK
    ac_d = nc.declare_dram_parameter("acat", [2 * M, nchunk, CCW], F16, isOutput=False)
    w_d = nc.declare_dram_parameter("aghw", [128, 64 * 128], F16, isOutput=False)
    out_d = nc.declare_dram_parameter("out", [nlp, A * G + A * H], F32, isOutput=True)

    Sq = mybir.ActivationFunctionType.Square
    ngroups = nlp // NB2
    gp = NB2 // 2  # pairs per group (256)
    odv = out_d[:, :].rearrange("(eo q) c -> q eo c", eo=2)

    with tile.TileContext(nc) as tc:
        with (
            tc.tile_pool(name="singles", bufs=1) as singles,
            tc.tile_pool(name="vbig", bufs=2) as vbig_pool,
            tc.tile_pool(name="sq", bufs=2) as sq_pool,
            tc.tile_pool(name="vout", bufs=3) as vout_pool,
            tc.tile_pool(name="psum1", bufs=2, space="PSUM") as p1_pool,
            tc.tile_pool(name="psum2", bufs=3, space="PSUM") as p2_pool,
        ):
            aghw = singles.tile([128, 64 * 128], F16)
            nc.sync.dma_start(out=aghw[:, :], in_=w_d[:, :])

            stg = [
                singles.tile([128, CCW], F16, name=f"stg{i}") for i in range(NST)
            ]
            for t in stg:
                nc.vector.memset(t[:, :], 0.0)

            def stage1_chunk(g2, ch, vbig):
                chunk = g2 * 8 + ch
                acs = stg[chunk % NST]
                # even rows 0-47: [G-row-e 4096 | a_e 2048]
                nc.sync.dma_start(out=acs[0:M, 0:CCW], in_=ac_d[0:M, chunk, :])
                # odd rows 64-111: [G-row-o 4096 | a_o 2048]
                nc.scalar.dma_start(
                    out=acs[64 : 64 + M, 0:CCW], in_=ac_d[M : 2 * M, chunk, :]
                )
                vbv = vbig[:, :].rearrange("p (a n) -> p n a", a=A)
                for fl in range(4):
                    psum1 = p1_pool.tile([128, 512], F32)
                    for jj in range(NFL):
                        j = fl * NFL + jj
                        nc.tensor.matmul(
                            out=psum1[:, jj * A : (jj + 1) * A],
                            lhsT=acs[:, j * 128 : (j + 1) * 128],
                            rhs=acs[:, 4096 + j * A : 4096 + (j + 1) * A],
                            start=True,
                            stop=True,
                        )
                    n0 = ch * 32 + fl * NFL
                    nc.vector.tensor_copy(
                        out=vbv[:, n0 : n0 + NFL, :],
                        in_=psum1[:, :].rearrange("p (n a) -> p n a", a=A),
                    )

            def stage2_pass(B, C, vbig, vout):
                psum2 = p2_pool.tile([128, 1024], F32)
                for c in range(8):
                    a_ch = C * 8 + c
                    nc.tensor.matmul(
                        out=psum2[:, c * 128 : (c + 1) * 128],
                        lhsT=vbig[
                            :, a_ch * 256 + B * 128 : a_ch * 256 + B * 128 + 128
                        ],
                        rhs=aghw[:, a_ch * 128 : (a_ch + 1) * 128],
                        start=True,
                        stop=True,
                    )
                # square d bands (s=1..3) into f32, dims (c, eoh, s);
                # keep APs at <=3 free dims (BIR limit is 4 incl partition)
                sq = sq_pool.tile([128, 768], F32)
                p2v = psum2[:, :].rearrange(
                    "p (c s eh) -> p c eh s", c=8, s=4, eh=32
                )
                nc.scalar.activation(
                    out=sq[:, :].rearrange("p (c eh s) -> p c eh s", c=8, eh=32),
                    in_=p2v[:, :, :, 1:4],
                    func=Sq,
                )
                vov = vout[:, :].rearrange(
                    "p (eo r cc h) -> p r cc eo h", eo=2, r=2, cc=64
                )
                # S band passthrough (f32)
                nc.scalar.copy(
                    out=vov[:, 0, C * 8 : C * 8 + 8, :, :],
                    in_=psum2[:, :].rearrange(
                        "p (c s eo h) -> p c s eo h", c=8, s=4, eo=2
                    )[:, :, 0, :, :],
                )
                # sum the three squared d components
                nc.vector.reduce_sum(
                    out=vov[:, 1, C * 8 : C * 8 + 8, :, :],
                    in_=sq[:, :].rearrange("p (f s) -> p f s", s=3),
                    axis=mybir.AxisListType.X,
                )

            def flush_vout(g2p, B, vout):
                r0 = g2p * gp + B * 128
                nc.gpsimd.dma_start(
                    out=odv[r0 : r0 + 128, :, :],
                    in_=vout[:, :].rearrange("p (eo c) -> p eo c", eo=2),
                )

            for rep in range(reps):
                prev = None
                for g2 in range(ngroups + 1):
                    cur = None
                    if g2 < ngroups:
                        vbig = vbig_pool.tile([128, gp * A], F16)
                        cur = (g2, vbig)
                    vout_b = None
                    for step in range(8):
                        if cur is not None:
                            stage1_chunk(g2, step, cur[1])
                        if prev is not None:
                            for half_step in range(2):
                                idx = 2 * step + half_step
                                B, C = idx // 8, idx % 8
                                if C == 0:
                                    vout_b = vout_pool.tile([128, 4096], F32)
                                stage2_pass(B, C, prev[1], vout_b)
                                if C == 7:
                                    flush_vout(prev[0], B, vout_b)
                    prev = cur
    nc.compile()
    return nc


def _get_nc():
    if "nc" not in _nc_cache:
        _nc_cache["nc"] = _build()
    return _nc_cache["nc"]


def _prep_core(a, gs, gv, nlp=NLP):
    """[nl, M, *] fp32 slices -> chunk-major fp16 [2M, npair//32, 6144].

    Atom j < nlp/2 is the even member of pair j, atom nlp/2 + j the odd
    member. Rows 0:M = even payload, M:2M = odd payload, each
    [G-region 4096 | A 2048]. G pair block cols = (sec4, eo2, g16) with
    the opposite-parity 16-col slots zeroed.
    """
    nl = a.shape[0]
    half = nlp // 2
    nchunk = half // NCHK
    gblk = np.zeros((nlp, M, 4, G), np.float16)
    gblk[:nl, :, 0, :] = gs
    for d in range(3):
        gblk[:nl, :, 1 + d, :] = gv[:, :, :, d]
    aat = np.zeros((nlp, M, A), np.float16)
    aat[:nl] = a
    ge = np.transpose(gblk[:half], (1, 0, 2, 3)).reshape(M, nchunk, NCHK, 4, G)
    go = np.transpose(gblk[half:], (1, 0, 2, 3)).reshape(M, nchunk, NCHK, 4, G)
    ae = np.transpose(aat[:half], (1, 0, 2)).reshape(M, nchunk, NCHK * A)
    ao = np.transpose(aat[half:], (1, 0, 2)).reshape(M, nchunk, NCHK * A)
    acat = np.zeros((2 * M, nchunk, CCW), np.float16)
    gv_e = acat[0:M, :, 0:4096].reshape(M, nchunk, NCHK, 4, 2, G)
    gv_e[:, :, :, :, 0, :] = ge
    gv_o = acat[M : 2 * M, :, 0:4096].reshape(M, nchunk, NCHK, 4, 2, G)
    gv_o[:, :, :, :, 1, :] = go
    acat[0:M, :, 4096:CCW] = ae
    acat[M : 2 * M, :, 4096:CCW] = ao
    return acat


def _prep_w(agh):
    """agh [A,G,H] fp32 -> aghw [128, 8192] fp16.

    Per channel ch (128-col block): full block-diagonal moving operand
    over (s, eo): row = 32s + 16eo + g, col = ch*128 + 32s2 + 16eo2 + h;
    nonzero iff s == s2 and eo == eo2; block = I16 for s=0 (S
    passthrough), agh[ch].T-layout [g, h] for s = 1..3.
    """
    ag = np.asarray(agh, np.float32).astype(np.float16)  # [A, G, H]
    # [s, eo, g, ch, s2, eo2, h]
    w = np.zeros((4, 2, G, A, 4, 2, H), np.float16)
    eye = np.eye(G, dtype=np.float16)
    agt = np.transpose(ag, (1, 0, 2))  # [G, A, H]
    for eo in range(2):
        w[0, eo, :, :, 0, eo, :] = eye[:, None, :]
        for s in range(1, 4):
            w[s, eo, :, :, s, eo, :] = agt
    return np.ascontiguousarray(w.reshape(128, A * 4 * 2 * H))


def kernel(a, gs, gv, agh):
    a = np.asarray(a, np.float32)
    gs = np.asarray(gs, np.float32)
    gv = np.asarray(gv, np.float32)
    aghw = _prep_w(agh)
    nc = _get_nc()
    in_maps = []
    for c in range(NCORES):
        sl = slice(c * NL, (c + 1) * NL)
        acat = _prep_core(a[sl], gs[sl], gv[sl])
        in_maps.append({"acat": acat, "aghw": aghw})
    res = run_bass_kernel_spmd(nc, in_maps, list(range(NCORES))).results
    return np.concatenate([res[c]["out"][:NL] for c in range(NCORES)], axis=0)
